# revision 2
# baseline (speedup 1.0000x reference)
"""Masked multi-head attention on 8 Trainium2 NeuronCores — v2.

Problem (hardcoded): x[4,2048,512] f32, mask[1,4,2048,2048] bool,
Wq/Wk/Wv[512,512] f32.  out = softmax(mask? -inf : scores/sqrt(128)) @ (xWv.T)
per head (8 heads of dim 64), merged to [4,2048,512] f32.
Sharding: core c handles batch b=c//2 and head-quad hg=c%2.

v2 vs baseline (all exact-bf16 on the value path; measured rates in ns):
- ACT exp measured 1060ns/tile -> 135.7us for all 128 tiles: THE bottleneck.
  Offload knobs: n_trick tiles/16 use the DVE Schraudolph bf16-bits exp
  (int16 affine on the PSUM scores + bitcast-bf16 mask multiply; ~1.8us/tile
  on DVE, ~2% sawtooth err diluted by sqrt(fraction)); n_gp masks/16 move
  the mask multiply to the idle GPSIMD engine (~2.1us/tile there).
- Mask multiply is ONE broadcast tensor_mul per tile ([p,2,512] view) not two.
- No PE mask injection (DVE/GPSIMD handle all masks).
"""

from collections import deque

import numpy as np
import ml_dtypes

import concourse.bass as bass
import concourse.mybir as mybir
import concourse.tile as tile
from concourse import bacc
from concourse.bass_utils import run_bass_kernel_spmd
from concourse.masks import make_identity

BF16 = mybir.dt.bfloat16
F32 = mybir.dt.float32
I16 = mybir.dt.int16
NPBF16 = ml_dtypes.bfloat16

B, N, C = 4, 2048, 512
H, D = 8, 64
TEMP = float((2.0 * D) ** 0.5)  # sqrt(128)
P = 128
NCORES = 8
HPC = H // 2          # 4 heads per core
DQ = HPC * D          # 256 projection cols per core
KT = N // P           # 16 k tiles
QB = N // 512         # 4 q blocks
VW = D + 1            # V width incl. ones column
AV_DEPTH = 3
# Schraudolph bf16-bits exp: bf16_bits(exp(s/TEMP)) ~ round(s*A + B) as int16
A_TRICK = 128.0 * float(np.log2(np.e)) / TEMP
B_TRICK = 16256.0 - 128.0 * 0.04303566


def _build_program(repeat=1, n_trick=0, n_gp=0, ablate=()):
    nc = bacc.Bacc(
        "TRN2",
        target_bir_lowering=False,
        debug=False,
        enable_asserts=False,
        num_devices=NCORES,
    )

    xT = nc.dram_tensor("xT", [C, N], BF16, kind="ExternalInput").ap()
    wqT = nc.dram_tensor("wqT", [C, DQ], BF16, kind="ExternalInput").ap()
    wkT = nc.dram_tensor("wkT", [C, DQ], BF16, kind="ExternalInput").ap()
    wvT = nc.dram_tensor("wvT", [C, DQ], BF16, kind="ExternalInput").ap()
    nmT = nc.dram_tensor("nmT", [N, N], BF16, kind="ExternalInput").ap()
    o = nc.dram_tensor("o", [N, DQ], F32, kind="ExternalOutput").ap()

    nm_view = nmT.rearrange("(t p) q -> p t q", p=P)  # [128, 16, 2048]

    with tile.TileContext(nc) as tc:
        with (
            tc.tile_pool(name="constp", bufs=1) as constp,
            tc.tile_pool(name="xp", bufs=2) as xp,
            tc.tile_pool(name="wp", bufs=1) as wp,
            tc.tile_pool(name="qkvp", bufs=2) as qkvp,
            tc.tile_pool(name="maskp",
                         bufs=4 if "slack2" in ablate else 3) as maskp,
            tc.tile_pool(name="workp",
                         bufs=10 if "slack2" in ablate else 8) as workp,
            tc.tile_pool(name="outp", bufs=4) as outp,
            tc.tile_pool(name="psp", bufs=2, space="PSUM") as psp,
            tc.tile_pool(name="projp", bufs=2, space="PSUM") as projp,
            tc.tile_pool(name="psot", bufs=1, space="PSUM") as psot,
        ):
            ident = constp.tile([P, P], F32)
            make_identity(nc, ident)
            for _ in range(repeat):
                _emit_body(nc, tc, xT, wqT, wkT, wvT, nm_view, o,
                           xp, wp, qkvp, maskp, workp, outp, psp, projp,
                           psot, ident, n_trick, n_gp, ablate)

    nc.compile()
    return nc


def _emit_body(nc, tc, xT, wqT, wkT, wvT, nm_view, o,
               xp, wp, qkvp, maskp, workp, outp, psp, projp, psot, ident,
               n_trick, n_gp, ablate):
    # ---- load inputs ----
    # interleave x/wq/wk chunk DMAs so proj group (q,0,0) can start after
    # chunk c arrives rather than after ALL x then ALL w
    xt, ws = [], {"q": [], "k": [], "v": []}
    wdrams = {"q": wqT, "k": wkT, "v": wvT}
    for c in range(4):
        t = xp.tile([P, N], BF16, name=f"xt{c}", tag=f"xt{c}")
        nc.sync.dma_start(out=t, in_=xT[c * P:(c + 1) * P, :])
        xt.append(t)
        for wname in ("q", "k"):
            w = wp.tile([P, DQ], BF16, name=f"w{wname}{c}", tag=f"w{wname}{c}")
            nc.sync.dma_start(out=w, in_=wdrams[wname][c * P:(c + 1) * P, :])
            ws[wname].append(w)
    for c in range(4):
        w = wp.tile([P, DQ], BF16, name=f"wv{c}", tag=f"wv{c}")
        nc.sync.dma_start(out=w, in_=wdrams["v"][c * P:(c + 1) * P, :])
        ws["v"].append(w)

    # ---- projections ----
    qt_sb = [qkvp.tile([P, N], BF16, name=f"qt_sb{m}", tag=f"qt{m}")
             for m in range(2)]
    kt_sb = [qkvp.tile([P, N], BF16, name=f"kt_sb{m}", tag=f"kt{m}")
             for m in range(2)]

    def qk_group(wname, m, nb):
        t = (qt_sb if wname == "q" else kt_sb)[m]
        ps = projp.tile([P, 512], F32, name="proj_ps", tag="pj")
        for c in range(4):
            nc.tensor.matmul(
                ps,
                lhsT=ws[wname][c][:, m * P:(m + 1) * P],
                rhs=xt[c][:, nb * 512:(nb + 1) * 512],
                start=(c == 0),
                stop=(c == 3),
            )
        nc.vector.tensor_copy(t[:, nb * 512:(nb + 1) * 512], ps)

    vext = qkvp.tile([P, KT * HPC * VW], BF16)
    nc.gpsimd.memset(vext, 1.0)

    def v_group(kti):
        ps = projp.tile([P, DQ], F32, name="v_ps", tag="pj")
        for c in range(4):
            nc.tensor.matmul(
                ps,
                lhsT=xt[c][:, kti * P:(kti + 1) * P],
                rhs=ws["v"][c],
                start=(c == 0),
                stop=(c == 3),
            )
        dst_view = vext[:, kti * HPC * VW:(kti + 1) * HPC * VW].rearrange(
            "p (h e) -> p h e", h=HPC
        )[:, :, 0:D]
        src_view = ps.rearrange("p (h e) -> p h e", h=HPC)
        nc.vector.tensor_copy(dst_view, src_view)

    qk_group("q", 0, 0)
    qk_group("k", 0, 0)
    prelude = deque()
    for spec in [("k", 0, 1), ("k", 0, 2), ("k", 0, 3),
                 ("q", 1, 0), ("k", 1, 0), ("k", 1, 1), ("k", 1, 2),
                 ("k", 1, 3),
                 ("q", 1, 1), ("q", 1, 2), ("q", 1, 3),
                 ("q", 0, 1), ("q", 0, 2), ("q", 0, 3)]:
        prelude.append(lambda spec=spec: qk_group(*spec))
    vqueue = deque(lambda kti=kti: v_group(kti) for kti in range(KT))

    # ---- attention (software-pipelined emission) ----
    av_queue = deque()
    epi_stages = deque()

    av_depth = 4 if "slack2" in ablate else AV_DEPTH

    def emit_slot():
        if vqueue:
            vqueue.popleft()()
        if prelude:
            prelude.popleft()()
        if len(av_queue) > av_depth:
            av_queue.popleft()()
        if epi_stages:
            epi_stages.popleft()()

    def make_epilogue(ot, m, qb):
        stages = []

        def copy_stage():
            ots = outp.tile([VW, 1024], F32, name="ots", tag="ots")
            if "epiact" in ablate:
                nc.scalar.copy(ots, ot)
            else:
                nc.vector.tensor_copy(ots, ot)
            stages.append(ots)  # [0]
        yield copy_stage

        def tr_stage(half):
            def f():
                ots = stages[0]
                if half == 0:
                    tr = psp.tile([P, 1024], F32, name="tr", tag="st")
                    stages.append(tr)  # [1]
                tr = stages[1]
                for j in range(half * 4, half * 4 + 4):
                    sl, hl = j // 2, j % 2
                    col = (j // 4) * 512 + (j % 4) * VW
                    nc.tensor.transpose(
                        tr[:, col:col + VW],
                        ots[:, hl * 512 + sl * P: hl * 512 + (sl + 1) * P],
                        ident[0:VW, 0:VW],
                    )
            return f
        yield tr_stage(0)
        yield tr_stage(1)

        def norm_stage():
            ob = outp.tile([P, 8 * D], F32, name="ob", tag="ob")
            rec = outp.tile([P, 8], F32, name="rec", tag="rec")
            tr = stages[1]
            trv = tr.rearrange("p (g je) -> p g je", g=2)[:, :, 0:4 * VW] \
                    .rearrange("p g (j e) -> p g j e", j=4)
            recv = rec.rearrange("p (g j) -> p g j", g=2)
            nc.vector.reciprocal(recv[:, :, :, None], trv[:, :, :, D:D + 1])
            obv = ob.rearrange("p (g j e) -> p g j e", g=2, e=D)
            recb = recv[:, :, :, None].broadcast_to([P, 2, 4, D])
            nc.vector.tensor_mul(obv, trv[:, :, :, 0:D], recb)
            stages.append(ob)
        yield norm_stage

        def dma_stage():
            ob = stages[-1]
            nc.sync.dma_start(
                out=o[qb * 512:(qb + 1) * 512, 2 * m * D:(2 * m + 2) * D]
                    .rearrange("(sl p) (hl d) -> p sl hl d", p=P, hl=2),
                in_=ob.rearrange("p (sl hl d) -> p sl hl d", sl=4, hl=2),
            )
        yield dma_stage

    # kti-index sets for the offload knobs, spread evenly
    def spread(n):
        if not n:
            return set()
        step = KT / n
        return {int(step * i + step / 2) for i in range(n)}

    trick_kti = spread(n_trick)
    gp_kti = spread(n_gp) - trick_kti if n_gp else set()

    for qb in range(QB):
        nm = maskp.tile([P, KT, 512], BF16, name="nm", tag="nm")
        nc.sync.dma_start(out=nm, in_=nm_view[:, :, qb * 512:(qb + 1) * 512])
        for m in range(2):
            ot = psot.tile([VW, 1024], F32, name="ot", tag="ot")
            for kti in range(KT):
                st = psp.tile([P, 1024], F32, name="st", tag="st")
                for hl in range(2):
                    nc.tensor.matmul(
                        st[:, hl * 512:(hl + 1) * 512],
                        lhsT=kt_sb[m][hl * D:(hl + 1) * D,
                                      kti * P:(kti + 1) * P],
                        rhs=qt_sb[m][hl * D:(hl + 1) * D,
                                     qb * 512:(qb + 1) * 512],
                        start=True,
                        stop=True,
                    )
                nmb = nm[:, kti, None, :].broadcast_to([P, 2, 512])
                if kti in trick_kti:
                    # one fused custom-DVE op: bits = (st*A + B) * nm -> int16;
                    # bitcast of the int16 bits IS the masked bf16 exp
                    t16 = workp.tile([P, 1024], I16, name="t16", tag="ex")
                    acc = workp.tile([P, 1], F32, name="acc", tag="acc")
                    nc.vector.affine_mul_reduce(
                        t16.rearrange("p (t q) -> p t q", t=2), acc,
                        st.rearrange("p (t q) -> p t q", t=2), nmb,
                        A_TRICK, B_TRICK)
                    ex = t16.bitcast(BF16)
                else:
                    ex = workp.tile([P, 1024], BF16, name="ex", tag="ex")
                    exv = ex.rearrange("p (t q) -> p t q", t=2)
                    nc.scalar.activation(
                        ex, st, mybir.ActivationFunctionType.Exp,
                        scale=1.0 / TEMP,
                    )
                    eng = nc.gpsimd if kti in gp_kti else nc.vector
                    eng.tensor_mul(exv, exv, nmb)

                def av_stage(ot=ot, ex=ex, kti=kti, m=m, qb=qb):
                    for hl in range(2):
                        h = 2 * m + hl
                        nc.tensor.matmul(
                            ot[:, hl * 512:(hl + 1) * 512],
                            lhsT=vext[:, (kti * HPC + h) * VW:
                                      (kti * HPC + h + 1) * VW],
                            rhs=ex[:, hl * 512:(hl + 1) * 512],
                            start=(kti == 0),
                            stop=(kti == KT - 1),
                        )
                    if kti == KT - 1:
                        epi_stages.extend(make_epilogue(ot, m, qb))
                av_queue.append(av_stage)
                emit_slot()

    while av_queue:
        av_queue.popleft()()
    while epi_stages:
        epi_stages.popleft()()


_NC_CACHE = {}


def _get_program(repeat=1, n_trick=0, n_gp=0, ablate=()):
    key = (repeat, n_trick, n_gp, tuple(ablate))
    if key not in _NC_CACHE:
        _NC_CACHE[key] = _build_program(repeat, n_trick, n_gp, tuple(ablate))
    return _NC_CACHE[key]


def _make_in_maps(x, mask, Wq, Wk, Wv):
    in_maps = []
    for core in range(NCORES):
        b, hg = core // 2, core % 2
        hsl = slice(hg * DQ, (hg + 1) * DQ)
        in_maps.append({
            "xT": np.ascontiguousarray(x[b].T).astype(NPBF16),
            "wqT": np.ascontiguousarray(Wq[hsl, :].T).astype(NPBF16),
            "wkT": np.ascontiguousarray(Wk[hsl, :].T).astype(NPBF16),
            "wvT": np.ascontiguousarray(Wv[hsl, :].T).astype(NPBF16),
            "nmT": np.ascontiguousarray((~mask[0, b]).T).astype(NPBF16),
        })
    return in_maps


def _assemble(results):
    out = np.empty((B, N, C), dtype=np.float32)
    for core in range(NCORES):
        b, hg = core // 2, core % 2
        out[b, :, hg * DQ:(hg + 1) * DQ] = results[core]["o"]
    return out


def run(x, mask, Wq, Wk, Wv, repeat=1, n_trick=0, n_gp=0, ablate=(),
        **spmd_kwargs):
    nc = _get_program(repeat, n_trick, n_gp, ablate=ablate)
    in_maps = _make_in_maps(
        np.asarray(x), np.asarray(mask), np.asarray(Wq), np.asarray(Wk),
        np.asarray(Wv))
    res = run_bass_kernel_spmd(nc, in_maps, list(range(NCORES)),
                               **spmd_kwargs)
    return _assemble(res.results), res


def kernel(x, mask, Wq, Wk, Wv):
    out, _ = run(x, mask, Wq, Wk, Wv)
    return out
